# revision 11
# baseline (speedup 1.0000x reference)
"""Trainium2 Bass kernel for nn_Decoder_36636071035490.

Reference computes, for workers i and task/edge (j,l):
    z = worker_feature @ W            # [2000, 1]
    p1 = sigmoid(z + b)
    p2 = (1 - p1) / 9
    P[i, j, l] = p1_i^tau_jl * p2_i^(1 - tau_jl)      # [2000, 5000, 10] f32

Identity used on device (exact in exact arithmetic):
    P[i, f] = exp(a_i * tau_f + c_i)
    a_i = (z_i + b) + ln 9            # since logit(sigmoid(x)) = x
    c_i = -ln(1 + exp(z_i + b)) - ln 9

Sharding: workers 2000 -> 8 cores x 250 (data parallel); W/b/task_feature
replicated. Per core the [250, 50000] slab is produced by broadcasting tau
across the 128 SBUF partitions (step-0 DMA) and a single ScalarE ACTIVATE
per slice: out[p, f] = Exp(a_p * tau[f] + c_p) using the per-partition
scale/bias operands of the activation instruction. No PE, no PSUM; the
kernel is output-DMA bound (~50 MB/core out + ~25 MB broadcast).
"""

import numpy as np

WORKERS = 2000
TASKS = 5000
ET = 10
AB = 64
NCORES = 8
WPC = WORKERS // NCORES  # 250 workers per core
F = TASKS * ET  # 50000 output cols per worker
NSL = 8  # tau slices
SL = F // NSL  # 6250 cols per slice
LN9 = float(np.log(9.0))

_CACHE = {}


def _build_nc():
    import concourse.bass as bass
    import concourse.mybir as mybir
    from concourse import bacc
    from concourse.tile import TileContext
    from contextlib import ExitStack

    f32 = mybir.dt.float32
    AF = mybir.ActivationFunctionType
    OP = mybir.AluOpType

    nc = bacc.Bacc("TRN2")
    wk = nc.dram_tensor("wk", [WPC, AB], f32, kind="ExternalInput")
    # task features pre-replicated across the 128 partitions (host-side
    # broadcast of the same values; avoids a step-0 replicating DMA which
    # lands all packets on SDMA engines 0/1)
    tfb = nc.dram_tensor("tfb", [128, F], f32, kind="ExternalInput")
    Wd = nc.dram_tensor("W", [AB, 1], f32, kind="ExternalInput")
    bd = nc.dram_tensor("b", [1], f32, kind="ExternalInput")
    out = nc.dram_tensor("out", [WPC, F], f32, kind="ExternalOutput")

    WT = [(0, 128), (128, WPC - 128)]  # (start, size) worker tiles

    with TileContext(nc) as tc, ExitStack() as ctx:
        const = ctx.enter_context(tc.tile_pool(name="const", bufs=1))
        taub_p = ctx.enter_context(tc.tile_pool(name="taubp", bufs=2))
        stage_p = ctx.enter_context(tc.tile_pool(name="stagep", bufs=3))

        # ---- per-worker scalars a (ACT scale) and c (ACT bias)
        Wb = const.tile([128, AB], f32, name="Wb")
        nc.sync.dma_start(
            out=Wb, in_=Wd[:].rearrange("a b -> b a").to_broadcast((128, AB))
        )
        bcol = const.tile([128, 1], f32, name="bcol")
        nc.sync.dma_start(out=bcol, in_=bd[:].to_broadcast((128, 1)))

        acol = [const.tile([128, 1], f32, name=f"acol{t}") for t in range(2)]
        ccol = [const.tile([128, 1], f32, name=f"ccol{t}") for t in range(2)]

        for t, (w0, M) in enumerate(WT):
            wkt = const.tile([M, AB], f32, name=f"wkt{t}")
            nc.sync.dma_start(out=wkt, in_=wk[w0 : w0 + M, :])
            wk2 = const.tile([M, AB], f32, name=f"wk2_{t}")
            nc.vector.tensor_copy(wk2, wkt)
            Wb2 = const.tile([M, AB], f32, name=f"Wb2_{t}")
            nc.vector.tensor_copy(Wb2, Wb[0:M, :])
            prod = const.tile([M, AB], f32, name=f"prod{t}")
            nc.vector.tensor_mul(prod, wk2, Wb2)
            zcol = const.tile([M, 1], f32, name=f"zcol{t}")
            nc.vector.reduce_sum(out=zcol, in_=prod, axis=mybir.AxisListType.X)
            # a = z + b + ln9
            nc.vector.tensor_scalar(
                out=acol[t][0:M, :],
                in0=zcol,
                scalar1=bcol[0:M, :],
                scalar2=LN9,
                op0=OP.add,
                op1=OP.add,
            )
            # c = -ln(1 + exp(z + b)) - ln9
            ecol = const.tile([M, 1], f32, name=f"ecol{t}")
            nc.scalar.activation(
                out=ecol, in_=zcol, func=AF.Exp, bias=bcol[0:M, :], scale=1.0
            )
            lcol = const.tile([M, 1], f32, name=f"lcol{t}")
            nc.scalar.activation(out=lcol, in_=ecol, func=AF.Ln, bias=1.0, scale=1.0)
            nc.vector.tensor_scalar(
                out=ccol[t][0:M, :],
                in0=lcol,
                scalar1=-1.0,
                scalar2=-LN9,
                op0=OP.mult,
                op1=OP.add,
            )

        # ---- main loop: load replicated tau slice (ACT HWDGE ring), one big
        # ACT per worker tile, store via SP HWDGE ring
        for s in range(NSL):
            taub = taub_p.tile([128, SL], f32, name="taub", tag="taub")
            nc.scalar.dma_start(out=taub, in_=tfb[:, s * SL : (s + 1) * SL])
            for t, (w0, M) in enumerate(WT):
                stg = stage_p.tile([128, SL], f32, name="stg", tag="stg")
                nc.scalar.activation(
                    out=stg[0:M, :],
                    in_=taub[0:M, :],
                    func=AF.Exp,
                    bias=ccol[t][0:M, :],
                    scale=acol[t][0:M, :],
                )
                nc.sync.dma_start(
                    out=out[w0 : w0 + M, s * SL : (s + 1) * SL], in_=stg[0:M, :]
                )

    nc.compile()
    return nc


def _get_nc():
    if "nc" not in _CACHE:
        _CACHE["nc"] = _build_nc()
    return _CACHE["nc"]


def _make_in_maps(inputs_arr, W, b):
    tau_flat = np.ascontiguousarray(
        inputs_arr[WORKERS:, :ET], dtype=np.float32
    ).reshape(F)
    tfb = np.ascontiguousarray(np.broadcast_to(tau_flat, (128, F)))
    W = np.ascontiguousarray(W, dtype=np.float32)
    b = np.ascontiguousarray(b, dtype=np.float32)
    return [
        {
            "wk": np.ascontiguousarray(inputs_arr[c * WPC : (c + 1) * WPC, :AB]),
            "tfb": tfb,
            "W": W,
            "b": b,
        }
        for c in range(NCORES)
    ]


def _run(inputs_arr, W, b, **kwargs):
    from concourse import bass_utils

    nc = _get_nc()
    in_maps = _make_in_maps(inputs_arr, W, b)
    return bass_utils.run_bass_kernel_spmd(
        nc, in_maps, core_ids=list(range(NCORES)), **kwargs
    )


def kernel(inputs, W, b):
    inputs_arr = np.asarray(inputs, dtype=np.float32)
    res = _run(inputs_arr, np.asarray(W), np.asarray(b))
    out = np.concatenate([r["out"] for r in res.results], axis=0)
    return out.reshape(WORKERS, TASKS, ET)


# revision 12
# speedup vs baseline: 3.0044x; 3.0044x over previous
"""Trainium2 Bass kernel for nn_Decoder_36636071035490.

Reference computes, for workers i and task/edge (j,l):
    z = worker_feature @ W            # [2000, 1]
    p1 = sigmoid(z + b)
    p2 = (1 - p1) / 9
    P[i, j, l] = p1_i^tau_jl * p2_i^(1 - tau_jl)      # [2000, 5000, 10] f32

Identity used on device (exact in exact arithmetic):
    P[i, f] = exp(a_i * tau_f + c_i)
    a_i = (z_i + b) + ln 9            # since logit(sigmoid(x)) = x
    c_i = -ln(1 + exp(z_i + b)) - ln 9

Sharding: workers 2000 -> 8 cores x 250 (data parallel); W/b/task_feature
replicated. Per core the [250, 50000] slab is produced by broadcasting tau
across the 128 SBUF partitions (step-0 DMA) and a single ScalarE ACTIVATE
per slice: out[p, f] = Exp(a_p * tau[f] + c_p) using the per-partition
scale/bias operands of the activation instruction. No PE, no PSUM; the
kernel is output-DMA bound (~50 MB/core out + ~25 MB broadcast).
"""

import numpy as np

WORKERS = 2000
TASKS = 5000
ET = 10
AB = 64
NCORES = 8
WPC = WORKERS // NCORES  # 250 workers per core
F = TASKS * ET  # 50000 output cols per worker
NSL = 8  # tau slices
SL = F // NSL  # 6250 cols per slice
LN9 = float(np.log(9.0))

_CACHE = {}


def _build_nc():
    import concourse.bass as bass
    import concourse.mybir as mybir
    from concourse import bacc
    from concourse.tile import TileContext
    from contextlib import ExitStack

    f32 = mybir.dt.float32
    AF = mybir.ActivationFunctionType
    OP = mybir.AluOpType

    nc = bacc.Bacc("TRN2")
    wk = nc.dram_tensor("wk", [WPC, AB], f32, kind="ExternalInput")
    # task features pre-replicated across the 128 partitions (host-side
    # broadcast of the same values; avoids a step-0 replicating DMA which
    # lands all packets on SDMA engines 0/1)
    tfb = nc.dram_tensor("tfb", [128, F], f32, kind="ExternalInput")
    Wd = nc.dram_tensor("W", [AB, 1], f32, kind="ExternalInput")
    bd = nc.dram_tensor("b", [1], f32, kind="ExternalInput")
    out = nc.dram_tensor("out", [WPC, F], f32, kind="ExternalOutput")

    # Overlapping worker tiles: DMA only spreads across all 16 SDMA engines
    # for full-128-partition transfers, so tile 2 covers workers 122..249 and
    # rows 122..127 are computed/stored twice with identical values.
    WT = [(0, 128), (WPC - 128, 128)]

    with TileContext(nc) as tc, ExitStack() as ctx:
        const = ctx.enter_context(tc.tile_pool(name="const", bufs=1))
        taub_p = ctx.enter_context(tc.tile_pool(name="taubp", bufs=2))
        stage_p = ctx.enter_context(tc.tile_pool(name="stagep", bufs=3))

        # ---- per-worker scalars a (ACT scale) and c (ACT bias)
        Wb = const.tile([128, AB], f32, name="Wb")
        nc.sync.dma_start(
            out=Wb, in_=Wd[:].rearrange("a b -> b a").to_broadcast((128, AB))
        )
        bcol = const.tile([128, 1], f32, name="bcol")
        nc.sync.dma_start(out=bcol, in_=bd[:].to_broadcast((128, 1)))

        acol = [const.tile([128, 1], f32, name=f"acol{t}") for t in range(2)]
        ccol = [const.tile([128, 1], f32, name=f"ccol{t}") for t in range(2)]

        for t, (w0, M) in enumerate(WT):
            wkt = const.tile([M, AB], f32, name=f"wkt{t}")
            nc.sync.dma_start(out=wkt, in_=wk[w0 : w0 + M, :])
            wk2 = const.tile([M, AB], f32, name=f"wk2_{t}")
            nc.vector.tensor_copy(wk2, wkt)
            Wb2 = const.tile([M, AB], f32, name=f"Wb2_{t}")
            nc.vector.tensor_copy(Wb2, Wb[0:M, :])
            prod = const.tile([M, AB], f32, name=f"prod{t}")
            nc.vector.tensor_mul(prod, wk2, Wb2)
            zcol = const.tile([M, 1], f32, name=f"zcol{t}")
            nc.vector.reduce_sum(out=zcol, in_=prod, axis=mybir.AxisListType.X)
            # a = z + b + ln9
            nc.vector.tensor_scalar(
                out=acol[t][0:M, :],
                in0=zcol,
                scalar1=bcol[0:M, :],
                scalar2=LN9,
                op0=OP.add,
                op1=OP.add,
            )
            # c = -ln(1 + exp(z + b)) - ln9
            ecol = const.tile([M, 1], f32, name=f"ecol{t}")
            nc.scalar.activation(
                out=ecol, in_=zcol, func=AF.Exp, bias=bcol[0:M, :], scale=1.0
            )
            lcol = const.tile([M, 1], f32, name=f"lcol{t}")
            nc.scalar.activation(out=lcol, in_=ecol, func=AF.Ln, bias=1.0, scale=1.0)
            nc.vector.tensor_scalar(
                out=ccol[t][0:M, :],
                in0=lcol,
                scalar1=-1.0,
                scalar2=-LN9,
                op0=OP.mult,
                op1=OP.add,
            )

        # ---- main loop: load replicated tau slice (ACT HWDGE ring), one big
        # ACT per worker tile, store via SP HWDGE ring
        for s in range(NSL):
            taub = taub_p.tile([128, SL], f32, name="taub", tag="taub")
            nc.scalar.dma_start(out=taub, in_=tfb[:, s * SL : (s + 1) * SL])
            for t, (w0, M) in enumerate(WT):
                stg = stage_p.tile([128, SL], f32, name="stg", tag="stg")
                nc.scalar.activation(
                    out=stg[0:M, :],
                    in_=taub[0:M, :],
                    func=AF.Exp,
                    bias=ccol[t][0:M, :],
                    scale=acol[t][0:M, :],
                )
                nc.sync.dma_start(
                    out=out[w0 : w0 + M, s * SL : (s + 1) * SL], in_=stg[0:M, :]
                )

    nc.compile()
    return nc


def _get_nc():
    if "nc" not in _CACHE:
        _CACHE["nc"] = _build_nc()
    return _CACHE["nc"]


def _make_in_maps(inputs_arr, W, b):
    tau_flat = np.ascontiguousarray(
        inputs_arr[WORKERS:, :ET], dtype=np.float32
    ).reshape(F)
    tfb = np.ascontiguousarray(np.broadcast_to(tau_flat, (128, F)))
    W = np.ascontiguousarray(W, dtype=np.float32)
    b = np.ascontiguousarray(b, dtype=np.float32)
    return [
        {
            "wk": np.ascontiguousarray(inputs_arr[c * WPC : (c + 1) * WPC, :AB]),
            "tfb": tfb,
            "W": W,
            "b": b,
        }
        for c in range(NCORES)
    ]


def _run(inputs_arr, W, b, **kwargs):
    from concourse import bass_utils

    nc = _get_nc()
    in_maps = _make_in_maps(inputs_arr, W, b)
    return bass_utils.run_bass_kernel_spmd(
        nc, in_maps, core_ids=list(range(NCORES)), **kwargs
    )


def kernel(inputs, W, b):
    inputs_arr = np.asarray(inputs, dtype=np.float32)
    res = _run(inputs_arr, np.asarray(W), np.asarray(b))
    out = np.concatenate([r["out"] for r in res.results], axis=0)
    return out.reshape(WORKERS, TASKS, ET)
